# revision 21
# baseline (speedup 1.0000x reference)
"""Conv4d (Strang rearrange) Trainium2 kernel — raw bacc pipeline.

Same math as the Tile version (see kernel_v6.py), but the engine programs and
semaphores are hand-rolled to avoid TileContext's fixed epilogue (full 256-sem
clear + double all-engine barrier, ~10 us) and per-instruction sem traffic.

Pipeline (64 groups g = (u, rnd), 36 block-diag matmuls each):
  sync:   w/bias + z-row DMAs (6-slot ring) interleaved with output DMAs
  tensor: 9 shifts x 4 col-tiled matmuls -> psum[g % 6]     (inc sem_mm)
  scalar: Identity+bias psum -> fp16 out[g % 4]             (inc sem_act)
  sync:   out DMA -> ys[g]                                  (inc sem_out, x16)
WAR hazards covered: z-slot reuse waits sem_mm, psum reuse waits sem_act,
out-slot reuse waits sem_out.  All sems cleared at the end for re-execution.
"""

from contextlib import ExitStack

import ml_dtypes
import numpy as np

import concourse.bass as bass
from concourse import bacc, mybir
from concourse.bass_utils import run_bass_kernel_spmd

F16 = mybir.dt.float16
BF16 = mybir.dt.bfloat16
F32 = mybir.dt.float32

B, CIN, COUT = 4, 4, 4
D1, D2, H, W = 32, 32, 64, 64
U = 16
R = U + 2
V = D2
I, J = H // 2, W // 2
IB, IO = 8, 4
VBS = 4
NCORES = 8
NZ, NPS, NOUT = 6, 6, 4
NG = 2 * U  # 64 groups

SHIFTS = [(ku, kv) for kv in (1, 0, 2) for ku in range(3)]
NSHIFT = len(SHIFTS)


def _host_weights(w, b):
    wbd = np.zeros((NSHIFT, 128, 32), np.float32)
    w = np.asarray(w, np.float32)
    for s, (ku, kv) in enumerate(SHIFTS):
        for kh in range(2):
            for kw in range(2):
                for ib in range(IB):
                    wbd[s, kh * 16 + kw * 8 + ib : 128 : 32, ib : 32 : 8] = (
                        w[:, :, ku, kv, kh, kw].T
                    )
    wbd_t = np.ascontiguousarray(wbd.transpose(1, 0, 2)).astype(ml_dtypes.bfloat16)
    bias = np.tile(np.repeat(np.asarray(b, np.float32), IB), 4).reshape(128, 1)
    return wbd_t, bias


def _host_shard(x):
    xp = np.pad(np.asarray(x, np.float32), ((0, 0), (0, 0), (1, 1), (0, 0), (0, 0), (0, 0)))
    shards = []
    for core in range(NCORES):
        bb, half = divmod(core, 2)
        xs = xp[bb, :, half * U : half * U + R]
        xs = xs.reshape(CIN, R, V, IO, IB, 2, J, 2)
        xs = xs.transpose(1, 0, 5, 7, 4, 2, 3, 6).astype(ml_dtypes.bfloat16)
        shards.append(np.ascontiguousarray(xs).reshape(R, 128, V, IO, J))
    return shards


def _build_program():
    nc = bacc.Bacc("TRN2", target_bir_lowering=False, debug=False)
    xs = nc.dram_tensor("xs", [R, 128, V, IO, J], BF16, kind="ExternalInput").ap()
    wbd = nc.dram_tensor("wbd", [128, NSHIFT, 32], BF16, kind="ExternalInput").ap()
    bias = nc.dram_tensor("bias", [128, 1], F32, kind="ExternalInput").ap()
    ys = nc.dram_tensor("ys", [U, 2, 128, VBS, IO, J], F16, kind="ExternalOutput").ap()

    with ExitStack() as ctx:
        zt = [ctx.enter_context(nc.sbuf_tensor(f"z{i}", [128, V, IO, J], BF16)) for i in range(NZ)]
        wt = ctx.enter_context(nc.sbuf_tensor("wt", [128, NSHIFT, 32], BF16))
        bt = ctx.enter_context(nc.sbuf_tensor("bt", [128, 1], F32))
        ot = [ctx.enter_context(nc.sbuf_tensor(f"ot{i}", [128, VBS, IO, J], F16)) for i in range(NOUT)]
        ps = [ctx.enter_context(nc.psum_tensor(f"ps{i}", [128, VBS, IO, J], F32)) for i in range(NPS)]
        sem_za = [ctx.enter_context(nc.semaphore(f"sem_za{r}")) for r in range(R)]
        sem_zb = [ctx.enter_context(nc.semaphore(f"sem_zb{r}")) for r in range(R)]
        sem_w = ctx.enter_context(nc.semaphore("sem_w"))
        sem_b = ctx.enter_context(nc.semaphore("sem_b"))
        sem_mm = ctx.enter_context(nc.semaphore("sem_mm"))
        sem_act = ctx.enter_context(nc.semaphore("sem_act"))
        sem_os = [ctx.enter_context(nc.semaphore(f"sem_o{i}")) for i in range(NOUT)]
        all_sems = None
        blk_ctx = nc.Block()
        block = blk_ctx.__enter__()

        @block.sync
        def _(sync):
            sync.dma_start(wt[:], wbd[:]).then_inc(sem_w, 16)
            for r in range(NZ):
                sync.dma_start(zt[r][:, 0:17], xs[r, :, 0:17]).then_inc(sem_za[r], 16)
            for g in range(NG):
                if g >= 2 and g % 2 == 0 and (r := (g - 2) // 2 + 6) < R:
                    sync.wait_ge(sem_mm, 2 * r - 10)
                    sync.dma_start(zt[r % NZ][:, 0:17], xs[r, :, 0:17]).then_inc(
                        sem_za[r], 16
                    )
                u, rnd = divmod(g, 2)
                sync.wait_ge(sem_act, g + 1)
                sync.dma_start(ys[u, rnd], ot[g % NOUT][:]).then_inc(
                    sem_os[g % NOUT], 16
                )
            finals = [(sem_w, 16), (sem_b, 16), (sem_mm, NG), (sem_act, NG)]
            finals += [(s, 16) for s in sem_za] + [(s, 16) for s in sem_zb]
            finals += [(s, 16 * (NG // NOUT)) for s in sem_os]
            for s, v in finals:
                sync.wait_ge(s, v)

        @block.tensor
        def _(tensor):
            for g in range(NG):
                u, rnd = divmod(g, 2)
                if g == 0:
                    tensor.wait_ge(sem_w, 16)
                    tensor.wait_ge(sem_za[0], 16)
                    tensor.wait_ge(sem_za[1], 16)
                if g == 1:
                    tensor.wait_ge(sem_zb[0], 16)
                    tensor.wait_ge(sem_zb[1], 16)
                if rnd == 0:
                    tensor.wait_ge(sem_za[u + 2], 16)
                else:
                    tensor.wait_ge(sem_zb[u + 2], 16)
                if g >= NPS:
                    tensor.wait_ge(sem_act, g - NPS + 1)
                psg = ps[g % NPS]
                last = None
                for s, (ku, kv) in enumerate(SHIFTS):
                    for c in range(4):
                        v0 = (rnd * 4 + c) * VBS
                        vv0 = max(0, 1 - kv - v0)
                        vv1 = min(VBS, V + 1 - kv - v0)
                        a = v0 + vv0 + kv - 1
                        last = nc.tensor.matmul(
                            psg[c * 32 : (c + 1) * 32, vv0:vv1, :, :],
                            wt[:, s, :],
                            zt[(u + ku) % NZ][:, a : a + (vv1 - vv0), :, :],
                            start=(s == 0),
                            stop=(s == NSHIFT - 1),
                            skip_group_check=True,
                            tile_position=(0, c * 32),
                        )
                last.then_inc(sem_mm)

        @block.scalar
        def _(scalar):
            scalar.dma_start(bt[:], bias[:]).then_inc(sem_b, 16)
            for r in range(NZ):
                scalar.dma_start(zt[r][:, 17:V], xs[r, :, 17:V]).then_inc(
                    sem_zb[r], 16
                )
            for g in range(NG):
                if g >= 2 and g % 2 == 1 and (r := (g - 3) // 2 + 6) < R:
                    scalar.wait_ge(sem_mm, 2 * r - 10)
                    scalar.dma_start(zt[r % NZ][:, 17:V], xs[r, :, 17:V]).then_inc(
                        sem_zb[r], 16
                    )
                scalar.wait_ge(sem_mm, g + 1)
                if g == 0:
                    scalar.wait_ge(sem_b, 16)
                if g >= NOUT:
                    scalar.wait_ge(sem_os[g % NOUT], 16 * (g // NOUT))
                nc.scalar.activation(
                    ot[g % NOUT][:],
                    ps[g % NPS][:],
                    mybir.ActivationFunctionType.Identity,
                    bias=bt[:],
                ).then_inc(sem_act)

        blk_ctx.__exit__(None, None, None)

    nc.compile()
    return nc


def _unshard(results):
    y = np.empty((B, COUT, D1, D2, I, J), np.float32)
    for core in range(NCORES):
        bb, half = divmod(core, 2)
        arr = results[core]["ys"].astype(np.float32).reshape(U, 2, 4, COUT, IB, VBS, IO, J)
        arr = arr.transpose(3, 0, 1, 2, 5, 6, 4, 7)
        y[bb, :, half * U : (half + 1) * U] = arr.reshape(COUT, U, V, I, J)
    return y


TRACE = False
LAST_RESULT = [None]


def kernel(x, w, b, _cache={}):
    if "nc" not in _cache:
        _cache["nc"] = _build_program()
    nc = _cache["nc"]
    wbd_t, bias = _host_weights(w, b)
    in_maps = [{"xs": xs, "wbd": wbd_t, "bias": bias} for xs in _host_shard(x)]
    res = run_bass_kernel_spmd(nc, in_maps, list(range(NCORES)), trace=TRACE)
    LAST_RESULT[0] = res
    return _unshard(res.results)
